# revision 2
# baseline (speedup 1.0000x reference)
"""3x3 median blur (zero padding) on (16, 3, 512, 512) f32 for 8 NeuronCores.

Sharding: batch dim 16 -> 2 per core; each core processes 6 images (2b x 3c).

Algorithm (per image, all compute on the DVE in bf16):
  - median9 = med3(max3(col mins), med3(col meds), min3(col maxes)) after a
    vertical sort3 per column; vertical pairs and horizontal pair stats are
    shared between adjacent windows (15 min/max element-ops per pixel).
  - bf16 end-to-end: the host rounds f32->bf16. Rounding is monotone, so
    median(round(x)) == round(median(x)); per-element error <= 2^-9 relative
    (measured Frobenius rel err 1.7e-3, well under the 2e-2 gate). 16-bit
    operands halve DMA bytes and enable the DVE packed 2x mode.
  - The DVE 2x mode needs innermost stride 1 on every operand, so the host
    packs each image row into parity planes ([odd cols | even cols]); with
    the on-chip zero padding this makes every horizontal window access a
    contiguous slice. Horizontal pair stats then cover 257 positions per
    row instead of 513, and the output (also plane-form) is re-interleaved
    on the host. Total DVE work: 30,776 elems/image/partition, every
    tensor_tensor eligible for the 16-bit 2x mode.

Layout per image on one core:
  - 512 rows split 4 per partition (128 partitions x 4 rows R).
  - tP [128, 6, 514]: per partition 6 plane-packed rows = 4 output rows +
    1 halo row above + 1 below; each row [E(257) | O(257)] where
    E[k] = padded col 2k (E[0] = zero pad), O[k] = padded col 2k+1
    (O[256] = zero pad).
"""

import numpy as np

B, C, H, W = 16, 3, 512, 512
N_CORES = 8
B_LOC = B // N_CORES          # 2 batches per core
IMGS = B_LOC * C              # 6 images per core
R = 4                         # output rows per partition (128*4 = 512)
WP = W + 2                    # padded row width (514)
HP = WP // 2                  # 257 per parity plane

_STATE = {}


def _mk_ap(base_ap, offset, pattern):
    """Clone an AP with a manual [step, count] pattern (element units)."""
    import concourse.mybir as mybir

    ap = base_ap.copy()
    ap.ap = mybir.VecI64Pair(pattern)
    ap.offset = offset
    return ap


def _build_nc():
    import concourse.bacc as bacc
    import concourse.mybir as mybir
    from concourse.tile import TileContext

    dt = mybir.dt.bfloat16
    Alu = mybir.AluOpType

    nc = bacc.Bacc("TRN2")
    x = nc.dram_tensor("x", [IMGS, H, W], dt, kind="ExternalInput")
    y = nc.dram_tensor("y", [IMGS, H, W], dt, kind="ExternalOutput")

    with TileContext(nc) as tc:
        TT = nc.vector.tensor_tensor

        with (
            tc.tile_pool(name="big", bufs=2) as big,
            tc.tile_pool(name="mid", bufs=1) as mid,
        ):
            for img in range(IMGS):
                xi = x[img]
                yi = y[img]

                # ---- load host-packed plane rows (1 halo above, 4 real, 1 below)
                tP = big.tile([128, 6, WP], dt, tag="tP")
                # zero pads: E[0] (padded col 0) and O[256] (padded col 513)
                nc.gpsimd.memset(tP[:, :, 0 : WP : WP - 1], 0.0)
                # zero halo rows (rows 0 and 5; halo DMAs overwrite all but
                # the image-top/bottom partitions)
                nc.gpsimd.memset(tP[:, 0:6:5, 1 : W + 1], 0.0)
                # central 4 rows: image row 4p+r -> tile row r+1
                nc.sync.dma_start(
                    out=tP[:, 1:5, 1 : W + 1],
                    in_=xi.rearrange("(p r) w -> p r w", p=128),
                )
                # halo above: image row 4p-1 -> tile row 0 (partitions 1..127)
                nc.sync.dma_start(out=tP[1:128, 0, 1 : W + 1], in_=xi[3 : H - 4 : 4, :])
                # halo below: image row 4p+4 -> tile row 5 (partitions 0..126)
                nc.sync.dma_start(out=tP[0:127, 5, 1 : W + 1], in_=xi[4 : H - 3 : 4, :])

                # ---- phase 1: vertical sort3 of rows (j, j+1, j+2), j=0..3.
                # Shared vertical pairs at tile rows (1,2) and (3,4):
                #   j=0: pair0 + c=row0   j=1: pair0 + c=row3
                #   j=2: pair1 + c=row2   j=3: pair1 + c=row5
                pvmin = mid.tile([128, 2, WP], dt, tag="pvmin")
                pvmax = mid.tile([128, 2, WP], dt, tag="pvmax")
                TT(out=pvmin[:], in0=tP[:, 1:5:2, :], in1=tP[:, 2:6:2, :], op=Alu.min)
                TT(out=pvmax[:], in0=tP[:, 1:5:2, :], in1=tP[:, 2:6:2, :], op=Alu.max)

                # merged combine over (g, r): output row j = g + 2r
                #   c row = 3g + 2r ; pair row = r (broadcast over g)
                s0 = big.tile([128, R, WP], dt, tag="s0")
                s1 = big.tile([128, R, WP], dt, tag="s1")
                s2 = big.tile([128, R, WP], dt, tag="s2")
                tq = mid.tile([128, R, WP], dt, tag="tq")
                c_ap = _mk_ap(
                    tP[:], 0, [[6 * WP, 128], [3 * WP, 2], [2 * WP, 2], [1, WP]]
                )
                pvmin_b = _mk_ap(
                    pvmin[:], 0, [[2 * WP, 128], [0, 2], [WP, 2], [1, WP]]
                )
                pvmax_b = _mk_ap(
                    pvmax[:], 0, [[2 * WP, 128], [0, 2], [WP, 2], [1, WP]]
                )

                def s_ap(tile):
                    return _mk_ap(
                        tile[:], 0, [[R * WP, 128], [WP, 2], [2 * WP, 2], [1, WP]]
                    )

                TT(out=s_ap(s0), in0=pvmin_b, in1=c_ap, op=Alu.min)
                TT(out=s_ap(s2), in0=pvmax_b, in1=c_ap, op=Alu.max)
                TT(out=s_ap(tq), in0=pvmax_b, in1=c_ap, op=Alu.min)
                TT(out=s_ap(s1), in0=pvmin_b, in1=s_ap(tq), op=Alu.max)

                # ---- phase 2: pair stats on parity planes [128, R, 257]
                # pair k = (E[k], O[k]) = (padded cols 2k, 2k+1)
                pA = mid.tile([128, R, HP], dt, tag="pA")
                pC = mid.tile([128, R, HP], dt, tag="pC")
                pBm = mid.tile([128, R, HP], dt, tag="pBm")
                pBM = mid.tile([128, R, HP], dt, tag="pBM")
                E = slice(0, HP)
                O = slice(HP, WP)
                TT(out=pA[:], in0=s0[:, :, E], in1=s0[:, :, O], op=Alu.max)
                TT(out=pC[:], in0=s2[:, :, E], in1=s2[:, :, O], op=Alu.min)
                TT(out=pBm[:], in0=s1[:, :, E], in1=s1[:, :, O], op=Alu.min)
                TT(out=pBM[:], in0=s1[:, :, E], in1=s1[:, :, O], op=Alu.max)

                # ---- combines -> plane-form [128, R, 2, 256]:
                # g=0 (even pixels c=2m, padded x=2m+1): pair[m] + third E[m+1]
                # g=1 (odd pixels c=2m+1, padded x=2m+2): pair[m+1] + third O[m]
                M = 256

                def pair_sel(tile):
                    return _mk_ap(
                        tile[:], 0, [[R * HP, 128], [HP, R], [1, 2], [1, M]]
                    )

                def third_sel(tile):
                    return _mk_ap(
                        tile[:], 1, [[R * WP, 128], [WP, R], [M, 2], [1, M]]
                    )

                A = mid.tile([128, R, 2, M], dt, tag="A")
                Cc = mid.tile([128, R, 2, M], dt, tag="Cc")
                t1 = mid.tile([128, R, 2, M], dt, tag="t1")
                u = mid.tile([128, R, 2, M], dt, tag="u")
                TT(out=A[:], in0=pair_sel(pA), in1=third_sel(s0), op=Alu.max)
                TT(out=Cc[:], in0=pair_sel(pC), in1=third_sel(s2), op=Alu.min)
                TT(out=t1[:], in0=pair_sel(pBM), in1=third_sel(s1), op=Alu.min)
                TT(out=t1[:], in0=pair_sel(pBm), in1=t1[:], op=Alu.max)  # B

                # ---- med3(A, B, C), stays in plane form; host interleaves
                out_t = big.tile([128, R, 2, M], dt, tag="out_t")
                TT(out=u[:], in0=A[:], in1=t1[:], op=Alu.min)
                TT(out=A[:], in0=A[:], in1=t1[:], op=Alu.max)    # v (in place)
                TT(out=Cc[:], in0=A[:], in1=Cc[:], op=Alu.min)   # w (in place)
                TT(out=out_t[:], in0=u[:], in1=Cc[:], op=Alu.max)

                # ---- store plane-form rows [even pixels (256) | odd (256)]
                nc.sync.dma_start(
                    out=yi.rearrange("(p r) w -> p r w", p=128),
                    in_=out_t[:],
                )
    nc.compile()
    return nc


def _to_bf16(x):
    import ml_dtypes

    return np.asarray(x, dtype=np.float32).astype(ml_dtypes.bfloat16)


def _pack_planes(xb):
    """[..., W] image rows -> [odd cols (256) | even cols (256)].

    With on-chip zero pads this lands rows as [E(257) | O(257)] where
    E/O are the even/odd PADDED columns.
    """
    return np.concatenate([xb[..., 1::2], xb[..., 0::2]], axis=-1)


def _unpack_planes(yp):
    """[..., 512] = [even pixels | odd pixels] -> interleaved row."""
    out = np.empty_like(yp)
    out[..., 0::2] = yp[..., : W // 2]
    out[..., 1::2] = yp[..., W // 2 :]
    return out


def make_in_maps(x):
    xb = _pack_planes(_to_bf16(x))
    return [
        {"x": np.ascontiguousarray(xb[i * B_LOC : (i + 1) * B_LOC].reshape(IMGS, H, W))}
        for i in range(N_CORES)
    ]


def _get_nc():
    if "nc" not in _STATE:
        _STATE["nc"] = _build_nc()
    return _STATE["nc"]


def kernel(x: np.ndarray) -> np.ndarray:
    from concourse.bass_utils import run_bass_kernel_spmd

    x = np.asarray(x)
    assert x.shape == (B, C, H, W), x.shape

    nc = _get_nc()
    in_maps = make_in_maps(x)
    core_ids = list(range(N_CORES))
    # The first execution on a cold device has returned stale data once;
    # run a warmup pass and keep the second result.
    if "warm" not in _STATE:
        run_bass_kernel_spmd(nc, in_maps, core_ids=core_ids)
        _STATE["warm"] = True
    res = run_bass_kernel_spmd(nc, in_maps, core_ids=core_ids)
    _STATE["last_results"] = res
    out = np.concatenate(
        [
            _unpack_planes(np.asarray(r["y"]).astype(np.float32)).reshape(
                B_LOC, C, H, W
            )
            for r in res.results
        ],
        axis=0,
    )
    return out
